# revision 39
# baseline (speedup 1.0000x reference)
"""DGCN diffusion-graph-conv kernel for 8 Trainium2 NeuronCores.

Math (per batch b):
    x_cat = concat(inputs, state_t, ones)      # [N, C+1]  (ones row folds bias)
    out_b = tanh( x_cat @ W0' + sum_s [ A_s @ Y1s + (2 A_s^2) @ Y2s ] )
  where (projection-first + Chebyshev expansion, spmm/proj commute):
    W0'  = W_m0 - W_m2 - W_m4 (+ bias row)     # folds the "-x0" terms
    Y1s  = x_cat @ W_{2s+1},  Y2s = x_cat @ W_{2s+2}     # [N, HID]
  A_s^2 is precomputed on the host (sparse-sparse product), which makes all
  four diffusion matmuls INDEPENDENT - no serial chain, no transposes.

Distribution: pure data-parallel over batch (2 batches per core, 8 cores),
no collectives.

Device dataflow (fp8 DoubleRow spmm, "orientation B"):
  - A_s entries are k/16 and (2A_s^2) entries are k/128 with k <= 16 -> all
    exactly representable in fp8e4 (verified zero cast error).  Y1/Y2 are
    both PROJECTED in fp8 DR (one 256-deep mm per tile instead of two bf16
    chunks) and stored fp8: Y feeds the output only through the A/(2A^2)
    terms (~7-14% of output magnitude), so the ~5.7% fp8 projection noise
    lands at 1.18e-2 total rel err vs the 2e-2 gate.  The dominant m0'
    projection must stay bf16 (fp8 there blows the gate: 4.2e-2).
  - diffusion passes run with perf_mode=DoubleRow: stationary = node-tile
    PAIRS of Y [128, 2, 128] fp8 (256-deep contraction), moving = A^T
    pair-blocks [128, 2, 512] fp8 streamed from HBM in 0.5 MB chunks with a
    16-deep prefetch ring (deep prefetch decouples the A-stream from the PE
    and removes the DMA/PE lockstep stalls).  Each (support, half, jj)
    QUARTER streams both of the support's matrices through 4 psum banks
    (one 32-matmul group per bank) from the shared 8-buf pool: bank reuse
    skips a full quarter, so evacs never stall the PE and the final tail
    is only 4 evacs deep.  (Splitting proj/diffusion into 4+4 psum pools
    regresses badly - quarters then reuse their own banks immediately.)
  - measured PE floor: every 512-col matmul issues at ~216 ns back-to-back
    (1 moving col/cycle at 2.4 GHz regardless of dtype/perf-mode; fp8 DR's
    2x comes from the 256-deep contraction).  1098 matmuls x 216 ns =
    237 us; this kernel measures ~271 us end-to-end (the rest: ~9 us NEFF
    engine-init preamble, ~2 us first-x wait, ~14 us tail/epilogue, ~9 us
    Y-proj evac-rate stalls + stream jitter).
  - scheduling details that mattered (each worth 5-20 us): x inputs loaded
    as quarter-tiles in proj consumption order; wc pre-cast to bf16 on
    host (the f32-load + device-cast chain gated the first matmul); final
    out-DMAs issued from the ACT queue (a sync-queue out-DMA waiting on
    tanh sems head-of-line-blocks the A-chunk issue stream behind it).
  - acc stays feature-major; host transposes the final [128, N] per batch.
"""

import numpy as np

import concourse.bass as bass
import concourse.bacc as bacc
import concourse.tile as tile
from concourse import mybir
from concourse.bass import ts
from concourse.bass_utils import run_bass_kernel_spmd

F32 = mybir.dt.float32
BF16 = mybir.dt.bfloat16
FP8 = mybir.dt.float8e4
Alu = mybir.AluOpType
Act = mybir.ActivationFunctionType
DR = mybir.MatmulPerfMode.DoubleRow

B, N, IN_DIM, HID = 16, 4096, 64, 128
C = IN_DIM + HID              # 192
CB = C + 1                    # +1 ones row (bias folding)
M = 5
DEG = 16
NNZ = N * DEG
N_CORES = 8
BL = B // N_CORES             # 2 batches per core
N_SUP = 2
W2 = BL * HID                 # 256
NT = N // 128                 # 32 node tiles
NQ = NT // 2                  # 16 node-tile pairs (DoubleRow contraction)
NOB = N // 512                # 8 output 512-blocks
NMAT = 2 * N_SUP              # A_0, 2A_0^2, A_1, 2A_1^2

_prog_cache: dict = {}


def _install_ntff_hook():
    """Benchmark-only: wire up the NTFF profile hook that bass_utils
    expects under axon when trace=True (the antenv.axon_hooks shim module
    is absent in this image), and stub out the S3 artifact upload."""
    import sys
    import types

    try:
        import antenv
        import concourse.bass_utils as bu

        bu.upload_artifacts = lambda tmpdir: "local://" + tmpdir
        if "antenv.axon_hooks" in sys.modules:
            return
        import trn_agent_boot.trn_boot as tb

        hook = tb._ntff_profile_via_ctypes("/opt/axon/libaxon_pjrt.so")
        mod = types.ModuleType("antenv.axon_hooks")
        mod.get_axon_ntff_profile_hook = lambda: hook
        mod.set_axon_ntff_profile_hook = lambda h: None
        sys.modules["antenv.axon_hooks"] = mod
        antenv.axon_hooks = mod
    except Exception as e:  # profiling is best-effort
        print(f"ntff hook install failed: {e}")


def _build_program(n_sup: int):
    nc = bacc.Bacc(
        "TRN2",
        target_bir_lowering=False,
        debug=False,
        enable_asserts=False,
        num_devices=N_CORES,
    )

    x0T_d = nc.dram_tensor("x0T", [BL, CB, N], BF16, kind="ExternalInput").ap()
    # fp8 DR-pair copies of x_cat and the Y-columns of wc, for the Y
    # projections only: x8[b, p, i, n] = x_cat[b, n, 128*i+p] (zero-padded
    # rows 193..255), wc8y[p, i, m] = wc[128*i+p, 128+m].  Y1/Y2 feed the
    # diffusion through the A/(2A^2) terms (~7-14% of output magnitude), so
    # fp8 projection noise (~5.7% of Y) lands well under the 2e-2 gate,
    # unlike fp8-projecting the dominant m0' term (keeps bf16 below).
    x8_d = nc.dram_tensor("x8", [BL, 128, 2, N], FP8, kind="ExternalInput").ap()
    wc8y_d = nc.dram_tensor("wc8y", [128, 2, 512], FP8, kind="ExternalInput").ap()
    # m0' columns only, bf16 host-cast
    wc_d = nc.dram_tensor("wc", [CB, HID], BF16, kind="ExternalInput").ap()
    # A^T pair-blocks, v in {A_0, 2A_0^2, A_1, 2A_1^2}:
    # a8[v, qq, p, ob, i2, i, n] = mat_v[ob*512+n, (2*(2qq+i2)+i)*128+p]
    a8_d = nc.dram_tensor(
        "a8", [NMAT, NQ // 2, 128, NOB, 2, 2, 512], FP8, kind="ExternalInput"
    ).ap()
    # feature-major: out[b, f, n] = acc^T (bf16); host upcasts + transposes
    out_d = nc.dram_tensor("out", [BL, 128, N], BF16, kind="ExternalOutput").ap()

    KCH = [(0, 128), (128, CB - 128)]
    kn1 = CB - 128

    with tile.TileContext(nc) as tc:
        with (
            tc.tile_pool(name="persist", bufs=1) as persist,
            tc.tile_pool(name="xstage", bufs=2) as xstage,
            tc.tile_pool(name="apool", bufs=16) as apool,
            tc.tile_pool(name="ostage", bufs=2) as ostage,
            tc.tile_pool(name="ps", bufs=4, space="PSUM") as psp,
        ):
            # ---------- weights ----------
            wc8y = persist.tile([128, 2, 512], FP8, tag="wc8y")
            nc.sync.dma_start(out=wc8y[:], in_=wc8y_d)
            # fp8 x-pairs, loaded right after wc8y so the first Y-proj
            # matmul starts as early as possible
            x8 = []
            for b in range(BL):
                xb = persist.tile([128, 2, N], FP8, tag=f"x8{b}", name=f"x8{b}")
                for qt in range(4):
                    nc.sync.dma_start(
                        out=xb[:, :, qt * 1024 : (qt + 1) * 1024],
                        in_=x8_d[b, :, :, qt * 1024 : (qt + 1) * 1024],
                    )
                x8.append(xb)
            wc_bf0 = persist.tile([128, HID], BF16, tag="wc0")
            nc.sync.dma_start(out=wc_bf0[:], in_=wc_d[0:128, :])
            wc_bf1 = persist.tile([128, HID], BF16, tag="wc1")
            nc.sync.dma_start(out=wc_bf1[:kn1, :], in_=wc_d[128:CB, :])
            wc_bf = [wc_bf0, wc_bf1]

            # ---------- x0T tiles (bf16, for the m0 projection only) ----------
            # DMAs are emitted mid-warmup (see warmup_interleave): issuing
            # them up front would head-of-line-delay the first A-chunk
            # issues on the qSP queue, stalling the warmup diffusion matmuls
            xq = [[None] * 2 for _ in range(BL)]
            for b in range(BL):
                for kc in range(2):
                    xq[b][kc] = persist.tile(
                        [128, N], BF16, tag=f"x{b}{kc}", name=f"x{b}{kc}"
                    )

            def load_x0T(b):
                nc.sync.dma_start(out=xq[b][0][:], in_=x0T_d[b, 0:128, :])
                nc.sync.dma_start(out=xq[b][1][:kn1, :], in_=x0T_d[b, 128:CB, :])

            # ---------- persistent tensors ----------
            # yq[k][:, t, b, s, :] = fp8(Y{k+1}s[t-tile, batch b])
            yq = [persist.tile([128, NT, BL, 2, 128], FP8, tag=f"y{k}", name=f"y{k}")
                  for k in range(2)]
            accT = persist.tile([128, BL, N], F32, tag="accT")

            # ---------- projections (emitted interleaved with the first
            # diffusion quarter below: the proj matmuls are evac-rate-bound
            # on the DVE/Scalar psum copies, so alternating them with
            # diffusion matmuls keeps the PE at the 216 ns floor) ----------
            def proj_tile(b, t):
                # Y projection, node-major fp8 DR: one mm per (tile, batch)
                pa = psp.tile([128, 512], F32, tag="pp")
                nc.tensor.matmul(
                    pa[:], lhsT=x8[b][:, :, t * 128 : (t + 1) * 128],
                    rhs=wc8y[:], start=True, stop=True, perf_mode=DR,
                )
                nc.vector.tensor_copy(out=yq[0][:, t, b], in_=pa[:, 0:256])
                nc.scalar.copy(out=yq[1][:, t, b], in_=pa[:, 256:512])

            def m0_pair(i):
                # m0' projection, feature-major bf16, seeds accT
                b, ob = i // NOB, i % NOB
                pm = psp.tile([128, 512], F32, tag="pp")
                for kc, (k0, kn) in enumerate(KCH):
                    nc.tensor.matmul(
                        pm[:],
                        lhsT=wc_bf[kc][:kn, :],
                        rhs=xq[b][kc][:kn, ob * 512 : (ob + 1) * 512],
                        start=(kc == 0), stop=(kc == 1),
                    )
                if ob % 2 == 0:
                    nc.vector.tensor_copy(out=accT[:, b, ts(ob, 512)], in_=pm[:])
                else:
                    nc.scalar.copy(out=accT[:, b, ts(ob, 512)], in_=pm[:])

            # ---------- fp8 DoubleRow diffusion passes ----------
            # v: 0 = A_0 (on Y1s0), 1 = 2A_0^2 (on Y2s0), 2 = A_1, 3 = 2A_1^2
            # QUARTER passes: each (s, half, jj) streams both v's of support
            # s through 4 psum banks (one 32-matmul group per bank).  Bank
            # reuse skips a full quarter, so evacs never stall the PE, and
            # the final tail is only 4 evacs deep.
            ots = {}

            def diff_quarter(s: int, half: int, jj: int, final: bool,
                             tag: str = "psd", interleave=None):
                ps = [
                    psp.tile([128, 512], F32, tag=tag,
                             name=f"ps_{s}{half}{jj}_{b}{j}")
                    for b in range(BL) for j in range(2)
                ]
                for vi in range(2):
                    v, k = 2 * s + vi, vi
                    for qq in range(NQ // 2):
                        if interleave is not None:
                            interleave(vi, qq)
                        at = apool.tile(
                            [128, 2, 2, 2, 512], FP8, tag="apool",
                            name=f"a_{v}{half}{jj}_{qq}",
                        )
                        nc.sync.dma_start(
                            out=at[:],
                            in_=a8_d[
                                v, qq, :,
                                4 * half + 2 * jj : 4 * half + 2 * jj + 2,
                            ],
                        )
                        for i2 in range(2):
                            q = 2 * qq + i2
                            for b in range(BL):
                                lhsT = yq[k][:, 2 * q : 2 * q + 2, b, s]
                                for j in range(2):
                                    nc.tensor.matmul(
                                        ps[b * 2 + j][:],
                                        lhsT=lhsT,
                                        rhs=at[:, j, i2],
                                        start=(vi == 0 and q == 0),
                                        stop=(vi == 1 and q == NQ - 1),
                                        perf_mode=DR,
                                    )
                for b in range(BL):
                    if final and jj == 0 and b not in ots.get(half, {}):
                        ots.setdefault(half, {})[b] = ostage.tile(
                            [128, 4, 512], BF16, tag="ostage",
                            name=f"ot_{half}_{b}",
                        )
                    for j in range(2):
                        ob = half * 4 + jj * 2 + j
                        nc.vector.tensor_tensor(
                            out=accT[:, b, ts(ob, 512)],
                            in0=ps[b * 2 + j][:],
                            in1=accT[:, b, ts(ob, 512)],
                            op=Alu.add,
                        )
                        if final:
                            nc.scalar.activation(
                                out=ots[half][b][:, jj * 2 + j],
                                in_=accT[:, b, ts(ob, 512)],
                                func=Act.Tanh,
                            )
                    # out-DMA per (half, b, jj), issued from the Act queue
                    # right after its tanhs: keeps the qSP queue free for the
                    # A-chunk stream (a sync-queue out-DMA waiting on tanh
                    # sems head-of-line-blocks all later A-chunk issues)
                    if final:
                        nc.scalar.dma_start(
                            out=out_d[
                                b, :,
                                half * 2048 + jj * 1024 :
                                half * 2048 + (jj + 1) * 1024,
                            ],
                            in_=ots[half][b][:, jj * 2 : jj * 2 + 2],
                        )

            # warmup: quarter (0,0,0) interleaved with the projections.
            # The proj matmuls are evac-rate-bound (DVE cast + Scalar copy
            # per psum; only those engines read PSUM), so alternating them
            # with diffusion matmuls keeps the PE near the 216 ns floor.
            # Proj rides tag "pp" (4 banks), quarters ride "psd"/"pp"
            # alternately so bank reuse always skips a full quarter.
            def warmup_interleave(vi, qq):
                if vi == 0:
                    # exactly the Y tiles this chunk's stationaries need
                    for t in range(4 * qq, 4 * qq + 4):
                        for b in range(BL):
                            proj_tile(b, t)
                    if qq == 5:
                        load_x0T(0)
                    elif qq == 6:
                        load_x0T(1)
                else:
                    for i in range(2 * qq, 2 * qq + 2):
                        m0_pair(i)

            qidx = 0
            for s in range(N_SUP):
                for half in range(2):
                    for jj in range(2):
                        diff_quarter(
                            s, half, jj, final=(s == N_SUP - 1),
                            tag="psd" if qidx % 2 == 0 else "pp",
                            interleave=warmup_interleave if qidx == 0 else None,
                        )
                        qidx += 1

    nc.compile()
    return nc


def _build_a8(sup_rows, sup_cols, sup_vals, n_sup):
    """Densify {A_s, 2A_s^2} into DoubleRow-friendly fp8 A^T pair-blocks.

    a8[v, qq, p, ob, i2, i, n] = mat_v[ob*512 + n, (2*(2qq+i2)+i)*128 + p];
    all values are k/16 (A) or k/128 (2A^2) with small k -> exact in fp8e4.
    """
    import ml_dtypes
    from scipy import sparse

    a8 = np.empty((NMAT, NQ // 2, 128, NOB, 2, 2, 512), dtype=ml_dtypes.float8_e4m3)
    for s in range(n_sup):
        sp = sparse.coo_matrix(
            (
                sup_vals[s].astype(np.float32),
                (sup_rows[s].astype(np.int64), sup_cols[s].astype(np.int64)),
            ),
            shape=(N, N),
        ).tocsr()
        sp2 = (sp @ sp) * 2.0
        for k, mat in enumerate((sp, sp2)):
            dense = np.asarray(mat.todense(), dtype=np.float32)
            # [ob, n, qq, i2, i, p] -> [qq, p, ob, i2, i, n]
            a7 = dense.reshape(NOB, 512, NQ // 2, 2, 2, 128)
            a8[2 * s + k] = a7.transpose(2, 5, 0, 3, 4, 1).astype(
                ml_dtypes.float8_e4m3
            )
    return a8


def _prep_core_inputs(inputs, state_t, weights, biases, sup_rows, sup_cols, sup_vals):
    """Host-side sharding: batch-parallel slices + layout prep."""
    import ml_dtypes

    w5 = weights.reshape(C, M, HID)
    wc = np.zeros((CB, M, HID), dtype=np.float32)
    # column order [m0', Y1s0, Y1s1, Y2s0, Y2s1]
    wc[:C, 0] = w5[:, 0] - w5[:, 2] - w5[:, 4]
    wc[C, 0] = biases.astype(np.float32)
    wc[:C, 1] = w5[:, 1]
    wc[:C, 2] = w5[:, 3]
    wc[:C, 3] = w5[:, 2]
    wc[:C, 4] = w5[:, 4]
    wc = np.ascontiguousarray(wc.reshape(CB, M * HID))
    wcm0 = wc[:, 0:HID]
    # wc8y[p, i, m] = wc[128*i + p, 128 + m] (Y columns, fp8 DR-pair layout)
    wcp = np.zeros((256, 512), dtype=np.float32)
    wcp[:CB] = wc[:, HID:]
    wc8y = np.ascontiguousarray(
        wcp.reshape(2, 128, 512).transpose(1, 0, 2)
    ).astype(ml_dtypes.float8_e4m3)

    a8 = _build_a8(sup_rows, sup_cols, sup_vals, N_SUP)

    in_maps = []
    for core in range(N_CORES):
        b0 = core * BL
        xcat = np.concatenate(
            [
                inputs[b0 : b0 + BL],
                state_t[b0 : b0 + BL],
                np.ones((BL, N, 1), dtype=np.float32),
            ],
            axis=2,
        )  # [BL, N, CB]
        x0T = np.ascontiguousarray(xcat.transpose(0, 2, 1)).astype(ml_dtypes.bfloat16)
        # x8[b, p, i, n] = x_cat[b, n, 128*i + p], zero pad c in [193, 256)
        xpad = np.zeros((BL, N, 256), dtype=np.float32)
        xpad[:, :, :CB] = xcat
        x8 = np.ascontiguousarray(
            xpad.reshape(BL, N, 2, 128).transpose(0, 3, 2, 1)
        ).astype(ml_dtypes.float8_e4m3)
        in_maps.append({"x0T": x0T, "x8": x8, "wc8y": wc8y,
                        "wc": wcm0.astype(ml_dtypes.bfloat16), "a8": a8})
    return in_maps


def _core_out_to_batches(o):
    """Device out [BL, 128, N] bf16 feature-major -> [N, HID] f32 per batch."""
    return [np.ascontiguousarray(o[b].T.astype(np.float32)) for b in range(BL)]


def kernel(
    inputs,
    state_t,
    weights,
    biases,
    sup_rows,
    sup_cols,
    sup_vals,
    _bench=None,
):
    inputs = np.asarray(inputs)
    state_t = np.asarray(state_t)
    weights = np.asarray(weights, dtype=np.float32)
    biases = np.asarray(biases, dtype=np.float32)
    sup_rows = np.asarray(sup_rows)
    sup_cols = np.asarray(sup_cols)
    sup_vals = np.asarray(sup_vals)

    if "prog" not in _prog_cache:
        _prog_cache["prog"] = _build_program(N_SUP)
    nc = _prog_cache["prog"]

    in_maps = _prep_core_inputs(
        inputs, state_t, weights, biases, sup_rows, sup_cols, sup_vals
    )
    trace = _bench is not None
    if trace:
        _install_ntff_hook()
    res = run_bass_kernel_spmd(nc, in_maps, list(range(N_CORES)), trace=trace)
    if _bench is not None:
        _bench["exec_time_ns"] = res.exec_time_ns
        _bench["mean_exec_time_ns"] = res.mean_exec_time_ns
        _bench["results"] = res

    out = np.empty((B, N, HID), dtype=np.float32)
    for core in range(N_CORES):
        o = res.results[core]["out"]  # [BL, 128, N]
        for b, ob in enumerate(_core_out_to_batches(np.asarray(o))):
            out[core * BL + b] = ob
    return out



# revision 40
# speedup vs baseline: 1.0085x; 1.0085x over previous
"""DGCN diffusion-graph-conv kernel for 8 Trainium2 NeuronCores.

Math (per batch b):
    x_cat = concat(inputs, state_t, ones)      # [N, C+1]  (ones row folds bias)
    out_b = tanh( x_cat @ W0' + sum_s [ A_s @ Y1s + (2 A_s^2) @ Y2s ] )
  where (projection-first + Chebyshev expansion, spmm/proj commute):
    W0'  = W_m0 - W_m2 - W_m4 (+ bias row)     # folds the "-x0" terms
    Y1s  = x_cat @ W_{2s+1},  Y2s = x_cat @ W_{2s+2}     # [N, HID]
  A_s^2 is precomputed on the host (sparse-sparse product), which makes all
  four diffusion matmuls INDEPENDENT - no serial chain, no transposes.

Distribution: pure data-parallel over batch (2 batches per core, 8 cores),
no collectives.

Device dataflow (fp8 DoubleRow spmm, "orientation B"):
  - A_s entries are k/16 and (2A_s^2) entries are k/128 with k <= 16 -> all
    exactly representable in fp8e4 (verified zero cast error).  Y1/Y2 are
    both PROJECTED in fp8 DR (one 256-deep mm per tile instead of two bf16
    chunks) and stored fp8: Y feeds the output only through the A/(2A^2)
    terms (~7-14% of output magnitude), so the ~5.7% fp8 projection noise
    lands at 1.18e-2 total rel err vs the 2e-2 gate.  The dominant m0'
    projection must stay bf16 (fp8 there blows the gate: 4.2e-2).
  - diffusion passes run with perf_mode=DoubleRow: stationary = node-tile
    PAIRS of Y [128, 2, 128] fp8 (256-deep contraction), moving = A^T
    pair-blocks [128, 2, 512] fp8 streamed from HBM in 0.5 MB chunks with a
    16-deep prefetch ring (deep prefetch decouples the A-stream from the PE
    and removes the DMA/PE lockstep stalls).  Each (support, half, jj)
    QUARTER streams both of the support's matrices through 4 psum banks
    (one 32-matmul group per bank) from the shared 8-buf pool: bank reuse
    skips a full quarter, so evacs never stall the PE and the final tail
    is only 4 evacs deep.  (Splitting proj/diffusion into 4+4 psum pools
    regresses badly - quarters then reuse their own banks immediately.)
  - measured PE floor: every 512-col matmul issues at ~216 ns back-to-back
    (1 moving col/cycle at 2.4 GHz regardless of dtype/perf-mode; fp8 DR's
    2x comes from the 256-deep contraction).  1098 matmuls x 216 ns =
    237 us; this kernel measures ~271 us end-to-end (the rest: ~9 us NEFF
    engine-init preamble, ~2 us first-x wait, ~14 us tail/epilogue, ~9 us
    Y-proj evac-rate stalls + stream jitter).
  - scheduling details that mattered (each worth 5-20 us): x inputs loaded
    as quarter-tiles in proj consumption order; wc pre-cast to bf16 on
    host (the f32-load + device-cast chain gated the first matmul); final
    out-DMAs issued from the ACT queue (a sync-queue out-DMA waiting on
    tanh sems head-of-line-blocks the A-chunk issue stream behind it).
  - acc stays feature-major; host transposes the final [128, N] per batch.
"""

import numpy as np

import concourse.bass as bass
import concourse.bacc as bacc
import concourse.tile as tile
from concourse import mybir
from concourse.bass import ts
from concourse.bass_utils import run_bass_kernel_spmd

F32 = mybir.dt.float32
BF16 = mybir.dt.bfloat16
FP8 = mybir.dt.float8e4
Alu = mybir.AluOpType
Act = mybir.ActivationFunctionType
DR = mybir.MatmulPerfMode.DoubleRow

B, N, IN_DIM, HID = 16, 4096, 64, 128
C = IN_DIM + HID              # 192
CB = C + 1                    # +1 ones row (bias folding)
M = 5
DEG = 16
NNZ = N * DEG
N_CORES = 8
BL = B // N_CORES             # 2 batches per core
N_SUP = 2
W2 = BL * HID                 # 256
NT = N // 128                 # 32 node tiles
NQ = NT // 2                  # 16 node-tile pairs (DoubleRow contraction)
NOB = N // 512                # 8 output 512-blocks
NMAT = 2 * N_SUP              # A_0, 2A_0^2, A_1, 2A_1^2

_prog_cache: dict = {}


def _install_ntff_hook():
    """Benchmark-only: wire up the NTFF profile hook that bass_utils
    expects under axon when trace=True (the antenv.axon_hooks shim module
    is absent in this image), and stub out the S3 artifact upload."""
    import sys
    import types

    try:
        import antenv
        import concourse.bass_utils as bu

        bu.upload_artifacts = lambda tmpdir: "local://" + tmpdir
        if "antenv.axon_hooks" in sys.modules:
            return
        import trn_agent_boot.trn_boot as tb

        hook = tb._ntff_profile_via_ctypes("/opt/axon/libaxon_pjrt.so")
        mod = types.ModuleType("antenv.axon_hooks")
        mod.get_axon_ntff_profile_hook = lambda: hook
        mod.set_axon_ntff_profile_hook = lambda h: None
        sys.modules["antenv.axon_hooks"] = mod
        antenv.axon_hooks = mod
    except Exception as e:  # profiling is best-effort
        print(f"ntff hook install failed: {e}")


def _build_program(n_sup: int):
    nc = bacc.Bacc(
        "TRN2",
        target_bir_lowering=False,
        debug=False,
        enable_asserts=False,
        num_devices=N_CORES,
    )

    x0T_d = nc.dram_tensor("x0T", [BL, CB, N], BF16, kind="ExternalInput").ap()
    # fp8 DR-pair copies of x_cat and the Y-columns of wc, for the Y
    # projections only: x8[b, p, i, n] = x_cat[b, n, 128*i+p] (zero-padded
    # rows 193..255), wc8y[p, i, m] = wc[128*i+p, 128+m].  Y1/Y2 feed the
    # diffusion through the A/(2A^2) terms (~7-14% of output magnitude), so
    # fp8 projection noise (~5.7% of Y) lands well under the 2e-2 gate,
    # unlike fp8-projecting the dominant m0' term (keeps bf16 below).
    x8_d = nc.dram_tensor("x8", [BL, 128, 2, N], FP8, kind="ExternalInput").ap()
    wc8y_d = nc.dram_tensor("wc8y", [128, 2, 512], FP8, kind="ExternalInput").ap()
    # m0' columns only, bf16 host-cast
    wc_d = nc.dram_tensor("wc", [CB, HID], BF16, kind="ExternalInput").ap()
    # A^T pair-blocks, v in {A_0, 2A_0^2, A_1, 2A_1^2}:
    # a8[v, qq, p, ob, i2, i, n] = mat_v[ob*512+n, (2*(2qq+i2)+i)*128+p]
    a8_d = nc.dram_tensor(
        "a8", [NMAT, NQ // 2, 128, NOB, 2, 2, 512], FP8, kind="ExternalInput"
    ).ap()
    # feature-major: out[b, f, n] = acc^T (bf16); host upcasts + transposes
    out_d = nc.dram_tensor("out", [BL, 128, N], BF16, kind="ExternalOutput").ap()

    KCH = [(0, 128), (128, CB - 128)]
    kn1 = CB - 128

    with tile.TileContext(nc) as tc:
        with (
            tc.tile_pool(name="persist", bufs=1) as persist,
            tc.tile_pool(name="xstage", bufs=2) as xstage,
            tc.tile_pool(name="apool", bufs=16) as apool,
            tc.tile_pool(name="ostage", bufs=2) as ostage,
            tc.tile_pool(name="ps", bufs=8, space="PSUM") as psp,
        ):
            # ---------- weights ----------
            wc8y = persist.tile([128, 2, 512], FP8, tag="wc8y")
            nc.sync.dma_start(out=wc8y[:], in_=wc8y_d)
            # fp8 x-pairs, loaded right after wc8y so the first Y-proj
            # matmul starts as early as possible
            x8 = []
            for b in range(BL):
                xb = persist.tile([128, 2, N], FP8, tag=f"x8{b}", name=f"x8{b}")
                for qt in range(4):
                    nc.sync.dma_start(
                        out=xb[:, :, qt * 1024 : (qt + 1) * 1024],
                        in_=x8_d[b, :, :, qt * 1024 : (qt + 1) * 1024],
                    )
                x8.append(xb)
            wc_bf0 = persist.tile([128, HID], BF16, tag="wc0")
            nc.sync.dma_start(out=wc_bf0[:], in_=wc_d[0:128, :])
            wc_bf1 = persist.tile([128, HID], BF16, tag="wc1")
            nc.sync.dma_start(out=wc_bf1[:kn1, :], in_=wc_d[128:CB, :])
            wc_bf = [wc_bf0, wc_bf1]

            # ---------- x0T tiles (bf16, for the m0 projection only) ----------
            # DMAs are emitted mid-warmup (see warmup_interleave): issuing
            # them up front would head-of-line-delay the first A-chunk
            # issues on the qSP queue, stalling the warmup diffusion matmuls
            xq = [[None] * 2 for _ in range(BL)]
            for b in range(BL):
                for kc in range(2):
                    xq[b][kc] = persist.tile(
                        [128, N], BF16, tag=f"x{b}{kc}", name=f"x{b}{kc}"
                    )

            for b in range(BL):
                nc.sync.dma_start(out=xq[b][0][:], in_=x0T_d[b, 0:128, :])
                nc.sync.dma_start(out=xq[b][1][:kn1, :], in_=x0T_d[b, 128:CB, :])

            # ---------- persistent tensors ----------
            # yq[k][:, t, b, s, :] = fp8(Y{k+1}s[t-tile, batch b])
            yq = [persist.tile([128, NT, BL, 2, 128], FP8, tag=f"y{k}", name=f"y{k}")
                  for k in range(2)]
            accT = persist.tile([128, BL, N], F32, tag="accT")

            # ---------- projections (emitted interleaved with the first
            # diffusion quarter below: the proj matmuls are evac-rate-bound
            # on the DVE/Scalar psum copies, so alternating them with
            # diffusion matmuls keeps the PE at the 216 ns floor) ----------
            def proj_tile(b, t):
                # Y projection, node-major fp8 DR: one mm per (tile, batch)
                pa = psp.tile([128, 512], F32, tag="ps")
                nc.tensor.matmul(
                    pa[:], lhsT=x8[b][:, :, t * 128 : (t + 1) * 128],
                    rhs=wc8y[:], start=True, stop=True, perf_mode=DR,
                )
                nc.vector.tensor_copy(out=yq[0][:, t, b], in_=pa[:, 0:256])
                nc.scalar.copy(out=yq[1][:, t, b], in_=pa[:, 256:512])

            def m0_pair(i):
                # m0' projection, feature-major bf16, seeds accT
                b, ob = i // NOB, i % NOB
                pm = psp.tile([128, 512], F32, tag="ps")
                for kc, (k0, kn) in enumerate(KCH):
                    nc.tensor.matmul(
                        pm[:],
                        lhsT=wc_bf[kc][:kn, :],
                        rhs=xq[b][kc][:kn, ob * 512 : (ob + 1) * 512],
                        start=(kc == 0), stop=(kc == 1),
                    )
                if ob % 2 == 0:
                    nc.vector.tensor_copy(out=accT[:, b, ts(ob, 512)], in_=pm[:])
                else:
                    nc.scalar.copy(out=accT[:, b, ts(ob, 512)], in_=pm[:])

            # ---------- fp8 DoubleRow diffusion passes ----------
            # v: 0 = A_0 (on Y1s0), 1 = 2A_0^2 (on Y2s0), 2 = A_1, 3 = 2A_1^2
            # QUARTER passes: each (s, half, jj) streams both v's of support
            # s through 4 psum banks (one 32-matmul group per bank).  Bank
            # reuse skips a full quarter, so evacs never stall the PE, and
            # the final tail is only 4 evacs deep.
            ots = {}

            def diff_quarter(s: int, half: int, jj: int, final: bool):
                ps = [
                    psp.tile([128, 512], F32, tag="ps",
                             name=f"ps_{s}{half}{jj}_{b}{j}")
                    for b in range(BL) for j in range(2)
                ]
                for vi in range(2):
                    v, k = 2 * s + vi, vi
                    for qq in range(NQ // 2):
                        at = apool.tile(
                            [128, 2, 2, 2, 512], FP8, tag="apool",
                            name=f"a_{v}{half}{jj}_{qq}",
                        )
                        nc.sync.dma_start(
                            out=at[:],
                            in_=a8_d[
                                v, qq, :,
                                4 * half + 2 * jj : 4 * half + 2 * jj + 2,
                            ],
                        )
                        for i2 in range(2):
                            q = 2 * qq + i2
                            for b in range(BL):
                                lhsT = yq[k][:, 2 * q : 2 * q + 2, b, s]
                                for j in range(2):
                                    nc.tensor.matmul(
                                        ps[b * 2 + j][:],
                                        lhsT=lhsT,
                                        rhs=at[:, j, i2],
                                        start=(vi == 0 and q == 0),
                                        stop=(vi == 1 and q == NQ - 1),
                                        perf_mode=DR,
                                    )
                for b in range(BL):
                    if final and jj == 0 and b not in ots.get(half, {}):
                        ots.setdefault(half, {})[b] = ostage.tile(
                            [128, 4, 512], BF16, tag="ostage",
                            name=f"ot_{half}_{b}",
                        )
                    for j in range(2):
                        ob = half * 4 + jj * 2 + j
                        nc.vector.tensor_tensor(
                            out=accT[:, b, ts(ob, 512)],
                            in0=ps[b * 2 + j][:],
                            in1=accT[:, b, ts(ob, 512)],
                            op=Alu.add,
                        )
                        if final:
                            nc.scalar.activation(
                                out=ots[half][b][:, jj * 2 + j],
                                in_=accT[:, b, ts(ob, 512)],
                                func=Act.Tanh,
                            )
                    # out-DMA per (half, b, jj), issued from the Act queue
                    # right after its tanhs: keeps the qSP queue free for the
                    # A-chunk stream (a sync-queue out-DMA waiting on tanh
                    # sems head-of-line-blocks all later A-chunk issues)
                    if final:
                        nc.scalar.dma_start(
                            out=out_d[
                                b, :,
                                half * 2048 + jj * 1024 :
                                half * 2048 + (jj + 1) * 1024,
                            ],
                            in_=ots[half][b][:, jj * 2 : jj * 2 + 2],
                        )

            for b in range(BL):
                for t in range(NT):
                    proj_tile(b, t)
            for i in range(2 * NOB):
                m0_pair(i)

            for s in range(N_SUP):
                for half in range(2):
                    for jj in range(2):
                        diff_quarter(s, half, jj, final=(s == N_SUP - 1))

    nc.compile()
    return nc


def _build_a8(sup_rows, sup_cols, sup_vals, n_sup):
    """Densify {A_s, 2A_s^2} into DoubleRow-friendly fp8 A^T pair-blocks.

    a8[v, qq, p, ob, i2, i, n] = mat_v[ob*512 + n, (2*(2qq+i2)+i)*128 + p];
    all values are k/16 (A) or k/128 (2A^2) with small k -> exact in fp8e4.
    """
    import ml_dtypes
    from scipy import sparse

    a8 = np.empty((NMAT, NQ // 2, 128, NOB, 2, 2, 512), dtype=ml_dtypes.float8_e4m3)
    for s in range(n_sup):
        sp = sparse.coo_matrix(
            (
                sup_vals[s].astype(np.float32),
                (sup_rows[s].astype(np.int64), sup_cols[s].astype(np.int64)),
            ),
            shape=(N, N),
        ).tocsr()
        sp2 = (sp @ sp) * 2.0
        for k, mat in enumerate((sp, sp2)):
            dense = np.asarray(mat.todense(), dtype=np.float32)
            # [ob, n, qq, i2, i, p] -> [qq, p, ob, i2, i, n]
            a7 = dense.reshape(NOB, 512, NQ // 2, 2, 2, 128)
            a8[2 * s + k] = a7.transpose(2, 5, 0, 3, 4, 1).astype(
                ml_dtypes.float8_e4m3
            )
    return a8


def _prep_core_inputs(inputs, state_t, weights, biases, sup_rows, sup_cols, sup_vals):
    """Host-side sharding: batch-parallel slices + layout prep."""
    import ml_dtypes

    w5 = weights.reshape(C, M, HID)
    wc = np.zeros((CB, M, HID), dtype=np.float32)
    # column order [m0', Y1s0, Y1s1, Y2s0, Y2s1]
    wc[:C, 0] = w5[:, 0] - w5[:, 2] - w5[:, 4]
    wc[C, 0] = biases.astype(np.float32)
    wc[:C, 1] = w5[:, 1]
    wc[:C, 2] = w5[:, 3]
    wc[:C, 3] = w5[:, 2]
    wc[:C, 4] = w5[:, 4]
    wc = np.ascontiguousarray(wc.reshape(CB, M * HID))
    wcm0 = wc[:, 0:HID]
    # wc8y[p, i, m] = wc[128*i + p, 128 + m] (Y columns, fp8 DR-pair layout)
    wcp = np.zeros((256, 512), dtype=np.float32)
    wcp[:CB] = wc[:, HID:]
    wc8y = np.ascontiguousarray(
        wcp.reshape(2, 128, 512).transpose(1, 0, 2)
    ).astype(ml_dtypes.float8_e4m3)

    a8 = _build_a8(sup_rows, sup_cols, sup_vals, N_SUP)

    in_maps = []
    for core in range(N_CORES):
        b0 = core * BL
        xcat = np.concatenate(
            [
                inputs[b0 : b0 + BL],
                state_t[b0 : b0 + BL],
                np.ones((BL, N, 1), dtype=np.float32),
            ],
            axis=2,
        )  # [BL, N, CB]
        x0T = np.ascontiguousarray(xcat.transpose(0, 2, 1)).astype(ml_dtypes.bfloat16)
        # x8[b, p, i, n] = x_cat[b, n, 128*i + p], zero pad c in [193, 256)
        xpad = np.zeros((BL, N, 256), dtype=np.float32)
        xpad[:, :, :CB] = xcat
        x8 = np.ascontiguousarray(
            xpad.reshape(BL, N, 2, 128).transpose(0, 3, 2, 1)
        ).astype(ml_dtypes.float8_e4m3)
        in_maps.append({"x0T": x0T, "x8": x8, "wc8y": wc8y,
                        "wc": wcm0.astype(ml_dtypes.bfloat16), "a8": a8})
    return in_maps


def _core_out_to_batches(o):
    """Device out [BL, 128, N] bf16 feature-major -> [N, HID] f32 per batch."""
    return [np.ascontiguousarray(o[b].T.astype(np.float32)) for b in range(BL)]


def kernel(
    inputs,
    state_t,
    weights,
    biases,
    sup_rows,
    sup_cols,
    sup_vals,
    _bench=None,
):
    inputs = np.asarray(inputs)
    state_t = np.asarray(state_t)
    weights = np.asarray(weights, dtype=np.float32)
    biases = np.asarray(biases, dtype=np.float32)
    sup_rows = np.asarray(sup_rows)
    sup_cols = np.asarray(sup_cols)
    sup_vals = np.asarray(sup_vals)

    if "prog" not in _prog_cache:
        _prog_cache["prog"] = _build_program(N_SUP)
    nc = _prog_cache["prog"]

    in_maps = _prep_core_inputs(
        inputs, state_t, weights, biases, sup_rows, sup_cols, sup_vals
    )
    trace = _bench is not None
    if trace:
        _install_ntff_hook()
    res = run_bass_kernel_spmd(nc, in_maps, list(range(N_CORES)), trace=trace)
    if _bench is not None:
        _bench["exec_time_ns"] = res.exec_time_ns
        _bench["mean_exec_time_ns"] = res.mean_exec_time_ns
        _bench["results"] = res

    out = np.empty((B, N, HID), dtype=np.float32)
    for core in range(N_CORES):
        o = res.results[core]["out"]  # [BL, 128, N]
        for b, ob in enumerate(_core_out_to_batches(np.asarray(o))):
            out[core * BL + b] = ob
    return out

